# revision 1
# baseline (speedup 1.0000x reference)
"""Erosion (5x5 sliding-window min, geodesic border pad 1e4) on TRN2.

Layout: partition p holds rows 8p-2 .. 8p+9 of one image as 12
free-dim segments (halo -2,-1 | main 0..7 | halo +8,+9), each 1028
cols (2-col pads). The 4-row halo is re-read from DRAM via strided,
partition-aligned DMAs (SBUF->SBUF partition-shifted copies measure
~50 GB/s and must be avoided). Vertical pass = 3 shifted
tensor_tensor(min) along the segment axis, horizontal pass = 3 shifted
TTs within segments — all free-dim, no cross-partition traffic.

Column chunks of 128 for intermediates; the final op of each chunk
writes into a full-width per-image output tile so the store is ONE
DMA of 4KB runs per image (per-chunk stores with 1KB runs cost ~105us
in HWDGE descriptor generation). Loads ride the sync queue, stores the
scalar queue; t1/t2 pools triple-buffered.
"""

import numpy as np

import concourse.bacc as bacc
import concourse.mybir as mybir
import concourse.tile as tile
from concourse.bass_utils import run_bass_kernel_spmd

B, H, W = 32, 1024, 1024
N_CORES = 8
PER_CORE = B // N_CORES     # 4 images per core
PX = 2
PAD_VAL = 1e4
F32 = mybir.dt.float32
MIN = mybir.AluOpType.min

KR = 8                      # output rows per partition (128*8 = 1024)
SEGS = KR + 2 * PX          # 12 segments per partition
WP = W + 2 * PX             # 1028 padded width
CW = 128                    # output cols per chunk
CWH = CW + 2 * PX           # 132
N_CC = W // CW              # 8

_CACHE = {}


def build_nc(repeat: int = 1):
    nc = bacc.Bacc("TRN2", debug=False, num_devices=N_CORES)
    x = nc.dram_tensor("mask", [PER_CORE, H, W], F32, kind="ExternalInput").ap()
    y = nc.dram_tensor("out", [PER_CORE, H, W], F32, kind="ExternalOutput").ap()

    with tile.TileContext(nc) as tc:
        with (
            tc.tile_pool(name="const", bufs=1) as cpool,
            tc.tile_pool(name="xp", bufs=2) as xpool,
            tc.tile_pool(name="t1", bufs=3) as t1p,
            tc.tile_pool(name="t2", bufs=3) as t2p,
            tc.tile_pool(name="vp", bufs=2) as vpool,
            tc.tile_pool(name="op", bufs=1) as opool,
        ):
            # 1e4 source for row-pad fills (memset can't start at
            # partition 127; DMA is exempt from start-partition rules)
            cpad = cpool.tile([128, 2 * WP], F32)
            nc.vector.memset(cpad[:, :], PAD_VAL)

            for rep in range(repeat):
                for img in range(PER_CORE):
                    xt = xpool.tile([128, SEGS * WP], F32, tag="x")
                    x3 = xt[:, :].rearrange("p (s c) -> p s c", s=SEGS)

                    # column pads (all segments)
                    nc.vector.memset(x3[:, :, 0:PX], PAD_VAL)
                    nc.vector.memset(x3[:, :, W + PX : WP], PAD_VAL)
                    # row pads: partition 0 segs 0,1 / partition 127 segs 10,11
                    nc.sync.dma_start(
                        out=x3[0:1, 0:PX, PX : W + PX], in_=cpad[0:1, 0 : 2 * W]
                    )
                    nc.sync.dma_start(
                        out=x3[127:128, KR + PX : SEGS, PX : W + PX],
                        in_=cpad[0:1, 0 : 2 * W],
                    )

                    # main rows: partition p segs 2..9 <- rows 8p..8p+7
                    nc.sync.dma_start(
                        out=x3[:, PX : PX + KR, PX : W + PX],
                        in_=x[img].rearrange("(p s) c -> p s c", s=KR),
                    )
                    # halo segs via strided row sampling
                    nc.sync.dma_start(
                        out=x3[1:128, 0:1, PX : W + PX],
                        in_=x[img, KR - PX : H - PX : KR, :].unsqueeze(1),
                    )
                    nc.sync.dma_start(
                        out=x3[1:128, 1:2, PX : W + PX],
                        in_=x[img, KR - 1 : H - 1 : KR, :].unsqueeze(1),
                    )
                    nc.sync.dma_start(
                        out=x3[0:127, KR + PX : KR + PX + 1, PX : W + PX],
                        in_=x[img, KR:H:KR, :].unsqueeze(1),
                    )
                    nc.sync.dma_start(
                        out=x3[0:127, KR + PX + 1 : SEGS, PX : W + PX],
                        in_=x[img, KR + 1 : H : KR, :].unsqueeze(1),
                    )

                    of = opool.tile([128, KR * W], F32, tag="o")
                    of3 = of[:, :].rearrange("p (s c) -> p s c", s=KR)

                    for cc in range(N_CC):
                        c0 = cc * CW
                        xs = x3[:, :, c0 : c0 + CWH]

                        w2 = t1p.tile([128, (SEGS - 1) * CWH], F32, tag="t1")
                        w2_3 = w2[:, :].rearrange("p (s c) -> p s c", s=SEGS - 1)
                        nc.vector.tensor_tensor(
                            out=w2_3[:, :, :],
                            in0=xs[:, 0 : SEGS - 1, :],
                            in1=xs[:, 1:SEGS, :],
                            op=MIN,
                        )
                        w4 = t2p.tile([128, (SEGS - 3) * CWH], F32, tag="t2")
                        w4_3 = w4[:, :].rearrange("p (s c) -> p s c", s=SEGS - 3)
                        nc.vector.tensor_tensor(
                            out=w4_3[:, :, :],
                            in0=w2_3[:, 0 : SEGS - 3, :],
                            in1=w2_3[:, 2 : SEGS - 1, :],
                            op=MIN,
                        )
                        v = vpool.tile([128, KR * CWH], F32, tag="v")
                        v3 = v[:, :].rearrange("p (s c) -> p s c", s=KR)
                        nc.vector.tensor_tensor(
                            out=v3[:, :, :],
                            in0=w4_3[:, 0:KR, :],
                            in1=xs[:, 2 * PX : SEGS, :],
                            op=MIN,
                        )

                        a = t1p.tile([128, KR * (CWH - 1)], F32, tag="t1")
                        a3 = a[:, :].rearrange("p (s c) -> p s c", s=KR)
                        nc.vector.tensor_tensor(
                            out=a3[:, :, :],
                            in0=v3[:, :, 0 : CWH - 1],
                            in1=v3[:, :, 1:CWH],
                            op=MIN,
                        )
                        bb = t2p.tile([128, KR * (CWH - 3)], F32, tag="t2")
                        b3 = bb[:, :].rearrange("p (s c) -> p s c", s=KR)
                        nc.vector.tensor_tensor(
                            out=b3[:, :, :],
                            in0=a3[:, :, 0 : CWH - 3],
                            in1=a3[:, :, 2 : CWH - 1],
                            op=MIN,
                        )
                        nc.vector.tensor_tensor(
                            out=of3[:, :, c0 : c0 + CW],
                            in0=b3[:, :, 0:CW],
                            in1=v3[:, :, 2 * PX : CWH],
                            op=MIN,
                        )

                    nc.scalar.dma_start(
                        out=y[img].rearrange("(p s) c -> p s c", s=KR),
                        in_=of3[:, :, :],
                    )

    nc.compile()
    return nc


def run(mask: np.ndarray, trace: bool = False):
    assert mask.shape == (B, 1, H, W), mask.shape
    in_dtype = mask.dtype
    mask4 = np.ascontiguousarray(
        mask.reshape(B, H, W).astype(np.float32, copy=False)
    )
    if "nc" not in _CACHE:
        _CACHE["nc"] = build_nc(1)
    nc = _CACHE["nc"]
    in_maps = [
        {"mask": mask4[i * PER_CORE : (i + 1) * PER_CORE]} for i in range(N_CORES)
    ]
    res = run_bass_kernel_spmd(nc, in_maps, list(range(N_CORES)), trace=trace)
    out = np.concatenate([res.results[i]["out"] for i in range(N_CORES)], axis=0)
    return out.reshape(B, 1, H, W).astype(in_dtype, copy=False), res


def kernel(mask: np.ndarray) -> np.ndarray:
    return run(mask)[0]



# revision 3
# speedup vs baseline: 1.6498x; 1.6498x over previous
"""Erosion (5x5 sliding-window min, geodesic border pad 1e4) on TRN2.

Layout: partition p holds rows 8p-2 .. 8p+9 of one image as 12 segments
in a bf16 tile xt16 [128, 12x1028] (2-col pads). Loads are gpsimd SWDGE
cast-DMAs (f32 DRAM -> bf16 SBUF): one partition-contiguous main DMA
(rows 8p..8p+7) plus two 2-row halo DMAs; row pads come from a 1e4
constant tile; column pads are memset once (xt16 is a single buffer).

All min ops run on DVE in bf16 at 2x mode except the parity-forced
odd-shift op, which is made the FINAL op so its 1x cost also performs
the bf16 -> f32 output cast:
  V: m2_s = min(x_s, x_{s+1}); m4_r = min(m2_r, m2_{r+2});
     v_r = min(m4_r, m2_{r+3})                       (all 2x)
  H: b2 = min(v, v>>2); e = min(b2, v>>4)            (2x)
     out = min(e, b2>>1)  -> f32                     (1x, odd shift)
Store is one partition-contiguous HWDGE DMA (32KB runs) per image on
the scalar queue; loads ride the gpsimd SWDGE queue -> three DMA paths
(gpsimd loads / scalar stores) stay off each other's FIFOs and the DVE
is the only significant compute engine.
"""

import numpy as np

import concourse.bacc as bacc
import concourse.mybir as mybir
import concourse.tile as tile
from concourse.bass_utils import run_bass_kernel_spmd

B, H, W = 32, 1024, 1024
N_CORES = 8
PER_CORE = B // N_CORES     # 4 images per core
PX = 2
PAD_VAL = 1e4
F32 = mybir.dt.float32
BF16 = mybir.dt.bfloat16
MIN = mybir.AluOpType.min

KR = 8                      # output rows per partition (128*8 = 1024)
SEGS = KR + 2 * PX          # 12 segments per partition
WP = W + 2 * PX             # 1028 padded width

_CACHE = {}


def build_nc(repeat: int = 1):
    nc = bacc.Bacc("TRN2", debug=False, num_devices=N_CORES)
    x = nc.dram_tensor("mask", [PER_CORE, H, W], F32, kind="ExternalInput").ap()
    y = nc.dram_tensor("out", [PER_CORE, H, W], F32, kind="ExternalOutput").ap()

    with tile.TileContext(nc) as tc:
        with (
            tc.tile_pool(name="const", bufs=1) as cpool,
            tc.tile_pool(name="x16", bufs=1) as x16p,
            tc.tile_pool(name="pa", bufs=1) as pap,
            tc.tile_pool(name="pb", bufs=1) as pbp,
            tc.tile_pool(name="pv", bufs=1) as pvp,
            tc.tile_pool(name="op", bufs=2) as opool,
        ):
            # 1e4 source for row-pad fills (memset can't start at
            # partition 127; DMA is exempt from start-partition rules)
            cpad = cpool.tile([128, 2 * WP], BF16)
            nc.vector.memset(cpad[:, :], PAD_VAL)

            xt16 = x16p.tile([128, SEGS * WP], BF16)
            x16 = xt16[:, :].rearrange("p (s c) -> p s c", s=SEGS)
            # column pads: constant across images, memset once
            nc.vector.memset(x16[:, :, 0:PX], PAD_VAL)
            nc.vector.memset(x16[:, :, W + PX : WP], PAD_VAL)

            for rep in range(repeat):
                for img in range(PER_CORE):
                    # main rows (cast f32->bf16 during DMA):
                    # partition p segs 2..9 <- rows 8p..8p+7
                    nc.gpsimd.dma_start(
                        out=x16[:, PX : PX + KR, PX : W + PX],
                        in_=x[img].rearrange("(p s) c -> p s c", s=KR),
                    )
                    # halo pairs (2 contiguous rows per partition)
                    nc.gpsimd.dma_start(
                        out=x16[1:128, 0:PX, PX : W + PX],
                        in_=x[img, KR - PX : H - PX, :].rearrange(
                            "(p s) c -> p s c", s=KR
                        )[:, 0:PX, :],
                    )
                    nc.gpsimd.dma_start(
                        out=x16[0:127, KR + PX : SEGS, PX : W + PX],
                        in_=x[img, KR:H, :].rearrange(
                            "(p s) c -> p s c", s=KR
                        )[:, 0:PX, :],
                    )
                    # row pads (full padded width; col-pad overlap is the
                    # same 1e4 value)
                    nc.sync.dma_start(
                        out=x16[0:1, 0:PX, :], in_=cpad[0:1, :]
                    )
                    nc.sync.dma_start(
                        out=x16[127:128, KR + PX : SEGS, :], in_=cpad[0:1, :]
                    )

                    # vertical pass (segment-axis shifts, all bf16 2x)
                    w2 = pap.tile([128, (SEGS - 1) * WP], BF16, tag="a")
                    w2_3 = w2[:, :].rearrange("p (s c) -> p s c", s=SEGS - 1)
                    nc.vector.tensor_tensor(
                        out=w2_3[:, :, :],
                        in0=x16[:, 0 : SEGS - 1, :],
                        in1=x16[:, 1:SEGS, :],
                        op=MIN,
                    )
                    m4 = pbp.tile([128, KR * WP], BF16, tag="b")
                    m4_3 = m4[:, :].rearrange("p (s c) -> p s c", s=KR)
                    nc.vector.tensor_tensor(
                        out=m4_3[:, :, :],
                        in0=w2_3[:, 0:KR, :],
                        in1=w2_3[:, 2 : KR + 2, :],
                        op=MIN,
                    )
                    v = pvp.tile([128, KR * WP], BF16, tag="v")
                    v3 = v[:, :].rearrange("p (s c) -> p s c", s=KR)
                    nc.vector.tensor_tensor(
                        out=v3[:, :, :],
                        in0=m4_3[:, :, :],
                        in1=w2_3[:, 3 : KR + 3, :],
                        op=MIN,
                    )

                    # horizontal pass
                    WB = WP - 2            # 1026 cols in b2
                    b2 = pap.tile([128, KR * WB], BF16, tag="a")
                    b2_3 = b2[:, :].rearrange("p (s c) -> p s c", s=KR)
                    nc.vector.tensor_tensor(
                        out=b2_3[:, :, :],
                        in0=v3[:, :, 0:WB],
                        in1=v3[:, :, 2:WP],
                        op=MIN,
                    )
                    e = pbp.tile([128, KR * W], BF16, tag="b")
                    e3 = e[:, :].rearrange("p (s c) -> p s c", s=KR)
                    nc.vector.tensor_tensor(
                        out=e3[:, :, :],
                        in0=b2_3[:, :, 0:W],
                        in1=v3[:, :, 2 * PX : WP],
                        op=MIN,
                    )
                    # final op: odd shift (1x) + bf16 -> f32 output cast
                    of = opool.tile([128, KR * W], F32, tag="o")
                    of3 = of[:, :].rearrange("p (s c) -> p s c", s=KR)
                    nc.vector.tensor_tensor(
                        out=of3[:, :, :],
                        in0=e3[:, :, :],
                        in1=b2_3[:, :, 1 : W + 1],
                        op=MIN,
                    )

                    nc.scalar.dma_start(
                        out=y[img].rearrange("(p s) c -> p s c", s=KR),
                        in_=of3[:, :, :],
                    )

    nc.compile()
    return nc


def run(mask: np.ndarray, trace: bool = False):
    assert mask.shape == (B, 1, H, W), mask.shape
    in_dtype = mask.dtype
    mask4 = np.ascontiguousarray(
        mask.reshape(B, H, W).astype(np.float32, copy=False)
    )
    if "nc" not in _CACHE:
        _CACHE["nc"] = build_nc(1)
    nc = _CACHE["nc"]
    in_maps = [
        {"mask": mask4[i * PER_CORE : (i + 1) * PER_CORE]} for i in range(N_CORES)
    ]
    res = run_bass_kernel_spmd(nc, in_maps, list(range(N_CORES)), trace=trace)
    out = np.concatenate([res.results[i]["out"] for i in range(N_CORES)], axis=0)
    return out.reshape(B, 1, H, W).astype(in_dtype, copy=False), res


def kernel(mask: np.ndarray) -> np.ndarray:
    return run(mask)[0]
